# revision 7
# baseline (speedup 1.0000x reference)
"""Grouped MoE MLP (SwiGLU) kernel for Trainium2, 8 NeuronCores.

Strategy (expert-parallel, host-side routing):
  Tokens arrive pre-sorted by expert with per-expert counts.  The host
  splits each expert's token block into shards of SLOT rows, assigns
  J = ceil(n_shards/8) shards to every core (padding short/missing
  shards with zero rows), and gathers the matching expert weights per
  (core, slot).  Every core then runs the identical program: for each
  of its J slots, a dense SwiGLU MLP of SLOT tokens with that slot's
  expert weights.  No device-side routing or collectives are needed.

  Layouts are transposed on the host so both GEMMs contract over the
  SBUF partition dimension with no on-chip transposes:
    GEMM1: out1^T[f, t] = sum_h W1[h, f] * x[t, h]   (h on partitions)
    SwiGLU on feature-partitioned tiles
    GEMM2: out^T[o, t]  = sum_f W2[f, o] * h[t, f]   (f on partitions)
"""

import math
from contextlib import ExitStack

import ml_dtypes
import numpy as np

P = 128
HIDDEN = 2048
INTER = 1408
GU = 2 * INTER            # 2816 = gate+up columns
KH = HIDDEN // P          # 16 k-tiles for GEMM1
KI = INTER // P           # 11 k-tiles for GEMM2 / gate-up pair blocks
MO = HIDDEN // P          # 16 output feature blocks
N_CORES = 8
NT = 384                  # tokens per chunk (matmul moving free dim)
SLOT = 768                # rows per weight slot

BF16 = ml_dtypes.bfloat16

_PROGRAM_CACHE: dict = {}


def _build_program(n_slots: int, slot_rows: int, nt: int):
    import concourse.mybir as mybir
    import concourse.tile as tile
    from concourse import bacc

    T = n_slots * slot_rows
    bf16 = mybir.dt.bfloat16
    f32 = mybir.dt.float32

    nc = bacc.Bacc(None, target_bir_lowering=False, debug=False)
    xT = nc.dram_tensor("xT", [P, KH, T], bf16, kind="ExternalInput")
    w1 = nc.dram_tensor("w1", [n_slots, P, KH, GU], bf16, kind="ExternalInput")
    w2 = nc.dram_tensor("w2", [n_slots, P, KI, HIDDEN], bf16, kind="ExternalInput")
    outT = nc.dram_tensor("outT", [P, MO, T], f32, kind="ExternalOutput")

    with tile.TileContext(nc) as tc, ExitStack() as ctx:
        w1_pool = ctx.enter_context(tc.tile_pool(name="w1p", bufs=1))
        w2_pool = ctx.enter_context(tc.tile_pool(name="w2p", bufs=1))
        x_pool = ctx.enter_context(tc.tile_pool(name="xp", bufs=2))
        h_pool = ctx.enter_context(tc.tile_pool(name="hp", bufs=2))
        g_pool = ctx.enter_context(tc.tile_pool(name="gp", bufs=2))
        o_pool = ctx.enter_context(tc.tile_pool(name="op", bufs=1))
        ps1 = ctx.enter_context(tc.tile_pool(name="ps1", bufs=2, space="PSUM"))
        ps2 = ctx.enter_context(tc.tile_pool(name="ps2", bufs=2, space="PSUM"))

        for s in range(n_slots):
            w1t = w1_pool.tile([P, KH, GU], bf16)
            nc.sync.dma_start(w1t[:], w1[s])
            w2t = w2_pool.tile([P, KI, HIDDEN], bf16)
            nc.sync.dma_start(w2t[:], w2[s])
            for c in range(slot_rows // nt):
                t0 = s * slot_rows + c * nt
                xt = x_pool.tile([P, KH, nt], bf16)
                nc.sync.dma_start(xt[:], xT[:, :, t0 : t0 + nt])
                ht = h_pool.tile([P, KI, nt], bf16)
                ot = o_pool.tile([P, MO, nt], f32)
                for mp in range(KI):
                    pg = ps1.tile([P, nt], f32)
                    pu = ps1.tile([P, nt], f32)
                    for k in range(KH):
                        nc.tensor.matmul(
                            pg[:],
                            w1t[:, k, mp * P : (mp + 1) * P],
                            xt[:, k, :],
                            start=(k == 0),
                            stop=(k == KH - 1),
                        )
                    for k in range(KH):
                        nc.tensor.matmul(
                            pu[:],
                            w1t[:, k, (KI + mp) * P : (KI + mp + 1) * P],
                            xt[:, k, :],
                            start=(k == 0),
                            stop=(k == KH - 1),
                        )
                    gt = g_pool.tile([P, nt], bf16)
                    nc.scalar.activation(
                        gt[:], pg[:], mybir.ActivationFunctionType.Silu
                    )
                    nc.vector.tensor_mul(ht[:, mp, :], gt[:], pu[:])
                for m in range(MO):
                    po = ps2.tile([P, nt], f32)
                    for k in range(KI):
                        nc.tensor.matmul(
                            po[:],
                            w2t[:, k, m * P : (m + 1) * P],
                            ht[:, k, :],
                            start=(k == 0),
                            stop=(k == KI - 1),
                        )
                    nc.vector.tensor_copy(ot[:, m, :], po[:])
                nc.sync.dma_start(outT[:, :, t0 : t0 + nt], ot[:])
    nc.compile()
    return nc


def _get_program(n_slots: int, slot_rows: int, nt: int):
    key = (n_slots, slot_rows, nt)
    if key not in _PROGRAM_CACHE:
        _PROGRAM_CACHE[key] = _build_program(n_slots, slot_rows, nt)
    return _PROGRAM_CACHE[key]


def _pack_w1(w: np.ndarray) -> np.ndarray:
    # [HIDDEN, GU] f32 -> [P, KH, GU] bf16 with row h = 128*k + p
    return np.ascontiguousarray(
        w.reshape(KH, P, GU).transpose(1, 0, 2).astype(BF16)
    )


def _pack_w2(w: np.ndarray) -> np.ndarray:
    # [INTER, HIDDEN] f32 -> [P, KI, HIDDEN] bf16 with row f = 128*k + p
    return np.ascontiguousarray(
        w.reshape(KI, P, HIDDEN).transpose(1, 0, 2).astype(BF16)
    )


def _run(
    hidden_states: np.ndarray,
    merged_gate_up_proj: np.ndarray,
    merged_down_proj: np.ndarray,
    num_tokens_per_expert: np.ndarray,
    trace: bool = False,
):
    from concourse.bass_utils import run_bass_kernel_spmd

    counts = [int(c) for c in np.asarray(num_tokens_per_expert)]
    n_experts = len(counts)
    offs = np.concatenate([[0], np.cumsum(counts)]).astype(int)
    total = int(offs[-1])

    # Shard each expert's token block into SLOT-row pieces.
    shards = []  # (expert, row0, nrows)
    for e in range(n_experts):
        r = 0
        while r < counts[e]:
            n = min(SLOT, counts[e] - r)
            shards.append((e, r, n))
            r += n
    n_slots = max(1, math.ceil(len(shards) / N_CORES))
    while len(shards) < N_CORES * n_slots:
        shards.append((0, 0, 0))
    T = n_slots * SLOT

    nc = _get_program(n_slots, SLOT, NT)

    w1_packed = [_pack_w1(merged_gate_up_proj[e]) for e in range(n_experts)]
    w2_packed = [_pack_w2(merged_down_proj[e]) for e in range(n_experts)]
    x_bf16 = hidden_states.astype(BF16)

    in_maps = []
    for r in range(N_CORES):
        core_shards = shards[r * n_slots : (r + 1) * n_slots]
        x_core = np.zeros((T, HIDDEN), dtype=BF16)
        for s, (e, r0, n) in enumerate(core_shards):
            if n:
                x_core[s * SLOT : s * SLOT + n] = x_bf16[
                    offs[e] + r0 : offs[e] + r0 + n
                ]
        # [T, HIDDEN] -> [P, KH, T] with column h = 128*k + p
        xT_core = np.ascontiguousarray(
            x_core.T.reshape(KH, P, T).transpose(1, 0, 2)
        )
        in_maps.append(
            {
                "xT": xT_core,
                "w1": np.stack([w1_packed[e] for (e, _, _) in core_shards]),
                "w2": np.stack([w2_packed[e] for (e, _, _) in core_shards]),
            }
        )

    res = run_bass_kernel_spmd(nc, in_maps, list(range(N_CORES)), trace=trace)

    out = np.empty((total, HIDDEN), dtype=np.float32)
    for r in range(N_CORES):
        # [P, MO, T] -> [T, HIDDEN] with column o = 128*m + p
        o_core = res.results[r]["outT"].transpose(2, 1, 0).reshape(T, HIDDEN)
        core_shards = shards[r * n_slots : (r + 1) * n_slots]
        for s, (e, r0, n) in enumerate(core_shards):
            if n:
                out[offs[e] + r0 : offs[e] + r0 + n] = o_core[
                    s * SLOT : s * SLOT + n
                ]
    return out, res


def kernel(**inputs) -> np.ndarray:
    return _run(**inputs, trace=False)[0]


def run_traced(**inputs):
    return _run(**inputs, trace=True)
